# revision 56
# baseline (speedup 1.0000x reference)
"""Trainium2 Bass kernel for nn_Decoder_30949534335472 (hazard-MLP decoder).

Math (per token, H=512):
    h2  = h @ W_h1.T + b_h1
    res = relu(h @ W_r.T + b_r)
    a0  = tanh(h2);            z0 = tanh(a0 @ W1p.T + b1p)
    a   = tanh(h2 + wt*t);     z  = tanh(a @ W1p.T + b1p)
    hazard    = W2p·(z - z0) + res*t          (b2p cancels)
    intensity = W2p·[(1-z^2) ⊙ (W1p @ ((1-a^2) ⊙ wt))] + res + 1e-6
where wt/W1p/b1p/W2p are the relu-masked hazard params (jax.grad replaced
by its closed form).

Sharding: pure data parallel, batch axis 16 -> 8 cores x 2.
Layout: feature-major ([feature 128-chunk, token] tiles); h cast to bf16 on
host, loaded transposed via DMA-transpose (all issued up front on sync).
The t-term of the t-path is folded into the PSUM accumulation as a K=1
matmul (wt ⊗ t), so h2 and u share one bank per chunk.
Engines: PE = 68 MMs + 12 col-tiled matvecs per 512-token tile; ACT = tanh
(+bias fused); DVE = fused gradient/epilogue ops; GpSimd = DMAs only.
"""

import sys

sys.path.insert(0, "/opt/trn_rl_repo")

from contextlib import ExitStack

import ml_dtypes
import numpy as np

import concourse.bass as bass
import concourse.tile as tile
from concourse import bacc, mybir
from concourse import bass_utils

F32 = mybir.dt.float32
BF16 = mybir.dt.bfloat16
AF = mybir.ActivationFunctionType
ALU = mybir.AluOpType

B, S, H = 16, 2048, 512
NCORES = 8
NTOK = (B // NCORES) * S  # 4096 tokens per core
T = 512                   # tokens per compute tile
NTILE = NTOK // T         # 8
TM = 2048                 # tokens per X-load macro slab
NMACRO = NTOK // TM       # 2
KC = H // 128             # 4 feature chunks
MINPOS = 1e-6

_NC_CACHE = None


def _build_body(ctx: ExitStack, tc: "tile.TileContext", io: dict):
    nc = tc.nc

    wpool = ctx.enter_context(tc.tile_pool(name="weights", bufs=1))
    sb = ctx.enter_context(tc.tile_pool(name="sb", bufs=2))
    ps = ctx.enter_context(tc.tile_pool(name="ps", bufs=2, space="PSUM"))

    # ---- weights: packed DMAs on gpsimd (SWDGE), off the transpose path ----
    # per-output-chunk whT tiles: mm1(m=0) only gates on its own 2 DMAs
    whTm = [wpool.tile([128, KC, 128], BF16, name=f"whTm{m}") for m in range(KC)]
    w1T_s = wpool.tile([128, KC, H], BF16)
    wmv_s = wpool.tile([128, KC, 3], BF16)   # cols: W_r, W2p, -W2p
    wvec = wpool.tile([128, 17], F32)        # bh1 | b1p | wt | -wt | b_r@[0,16]
    t_all = wpool.tile([1, NTOK], F32)
    # DMA engine split: gpsimd carries only tiny vectors (its
    # partition_broadcast serializes behind in-flight SWDGE transfers);
    # w1T rides the scalar engine's HWDGE (idle until the first ACT);
    # everything else is ordered on sync by when the PE needs it.
    nc.gpsimd.dma_start(t_all[:], io["t"][:])
    nc.gpsimd.dma_start(wvec[:], io["wvec"][:])
    nc.gpsimd.dma_start(wmv_s[:], io["wmv"][:])
    for k in range(KC):
        nc.scalar.dma_start(w1T_s[:, k], io["w1T"][:, k])


    bh1 = wvec[:, 0:KC]
    b1p = wvec[:, KC:2 * KC]
    wt = wvec[:, 2 * KC:3 * KC]
    nwt = wvec[:, 3 * KC:4 * KC]
    br = wvec[0:1, 16:17]

    # ---- X = h^T loads: plain contiguous DMAs (h pre-transposed on host) --
    # The DMA-transpose xbar path serializes (~52 GB/s, ~1.2us issue each);
    # with hbT = [KC, 128, NTOK] from the host these are ordinary strided
    # DMAs with 4KB-contiguous rows that spread across all queues.
    # Graded slab sizes, each split in 2 sub-DMAs so transfers parallelize
    # across HWDGE queues; tile 0's X lands first, then whTm, then the rest.
    Xt = {}  # (tile, chunk) -> AP [128, T]
    slabs = [(0, 512), (512, 512), (1024, 1024), (2048, 2048)]
    nslab = {512: 2, 1024: 1, 2048: 1}

    def emit_slab(start, size):
        for k in range(KC):
            Xm = sb.tile([128, size], BF16, tag=f"Xm{size}", bufs=nslab[size] * KC)
            h_ = size // 2
            nc.sync.dma_start(Xm[:, 0:h_], io["hbT"][k, :, start:start + h_])
            nc.sync.dma_start(Xm[:, h_:], io["hbT"][k, :, start + h_:start + size])
            for i in range(start // T, (start + size) // T):
                off = i * T - start
                Xt[(i, k)] = Xm[:, off:off + T]

    emit_slab(*slabs[0])
    for m in range(KC):
        nc.sync.dma_start(whTm[m][:, 0:2], io["whTm"][m, :, 0:2])
        nc.sync.dma_start(whTm[m][:, 2:4], io["whTm"][m, :, 2:4])
    for sl in slabs[1:]:
        emit_slab(*sl)

    # ---- output accumulators, one DMA each at the end ----
    out_hz = wpool.tile([1, NTOK], F32)
    out_it = wpool.tile([1, NTOK], F32)

    def stage_a(i):
        """mm1 (+bias-free h2 then u via K=1 wt*t MM), a0/a/sq/da, res mv."""
        st = {"i": i}
        Xs = [Xt[(i, k)] for k in range(KC)]
        tB = sb.tile([128, T], F32, tag="tB", bufs=2)
        nc.gpsimd.partition_broadcast(tB[:], t_all[0:1, i * T:(i + 1) * T])

        a0 = sb.tile([128, KC, T], BF16, tag="a0", bufs=2)
        a = sb.tile([128, KC, T], BF16, tag="a", bufs=2)
        da = sb.tile([128, KC, T], BF16, tag="da", bufs=2)
        sq = sb.tile([128, KC, T], BF16, tag="sq", bufs=2)

        for m in range(KC):
            pv1 = ps.tile([128, T], F32, tag="pv1", bufs=2)
            for k in range(KC):
                nc.tensor.matmul(
                    pv1[:],
                    lhsT=whTm[m][:, k],
                    rhs=Xs[k],
                    start=(k == 0),
                    stop=(k == KC - 1),
                )
            nc.scalar.activation(a0[:, m], pv1[:], AF.Tanh, bias=bh1[:, m:m + 1])
            # u = h2 + wt*t on DVE (PE is the bottleneck engine)
            u = sb.tile([128, T], F32, tag="u", bufs=2)
            nc.vector.scalar_tensor_tensor(
                u[:], in0=tB[:], scalar=wt[:, m:m + 1], in1=pv1[:],
                op0=ALU.mult, op1=ALU.add,
            )
            nc.scalar.activation(a[:, m], u[:], AF.Tanh, bias=bh1[:, m:m + 1])

        nc.vector.tensor_mul(sq[:], a[:], a[:])  # batched a^2
        for m in range(KC):
            # da = wt - wt*a^2  (= wt*(1-a^2))
            nc.vector.tensor_scalar(
                da[:, m], sq[:, m], nwt[:, m:m + 1], wt[:, m:m + 1],
                op0=ALU.mult, op1=ALU.add,
            )

        st.update(Xs=Xs, a0=a0, a=a, da=da)
        return st

    def stage_b1(st, off=0, size=T):
        """z-chains (k-interleaved, 3 concurrent banks) + gradient
        elementwise, over tokens [off, off+size) of the tile."""
        tsl = slice(off, off + size)
        z0 = sb.tile([128, KC, size], BF16, tag="z0", bufs=2)
        z = sb.tile([128, KC, size], BF16, tag="z", bufs=2)
        zq = sb.tile([128, KC, size], BF16, tag="zq", bufs=2)
        dzneg = sb.tile([128, KC, size], BF16, tag="dzneg", bufs=2)
        dzz = sb.tile([128, KC, size], BF16, tag="dzz", bufs=2)

        for m in range(KC):
            pv0 = ps.tile([128, T], F32, tag="pmm", bufs=4)
            pv = ps.tile([128, T], F32, tag="pmm", bufs=4)
            pg = ps.tile([128, T], F32, tag="pmm", bufs=4)
            msl = slice(m * 128, (m + 1) * 128)
            for k in range(KC):
                nc.tensor.matmul(
                    pv0[:, :size], lhsT=w1T_s[:, k, msl], rhs=st["a0"][:, k, tsl],
                    start=(k == 0), stop=(k == KC - 1),
                )
                nc.tensor.matmul(
                    pv[:, :size], lhsT=w1T_s[:, k, msl], rhs=st["a"][:, k, tsl],
                    start=(k == 0), stop=(k == KC - 1),
                )
                nc.tensor.matmul(
                    pg[:, :size], lhsT=w1T_s[:, k, msl], rhs=st["da"][:, k, tsl],
                    start=(k == 0), stop=(k == KC - 1),
                )
            nc.scalar.activation(
                z0[:, m], pv0[:, :size], AF.Tanh, bias=b1p[:, m:m + 1]
            )
            nc.scalar.activation(
                z[:, m], pv[:, :size], AF.Tanh, bias=b1p[:, m:m + 1]
            )
            nc.vector.tensor_mul(zq[:, m], z[:, m], z[:, m])
            # (z^2 - 1) * g   (negated dz; folded into -W2p in the matvec)
            nc.vector.scalar_tensor_tensor(
                dzneg[:, m], in0=zq[:, m], scalar=1.0, in1=pg[:, :size],
                op0=ALU.subtract, op1=ALU.mult,
            )
        nc.vector.tensor_sub(dzz[:], z[:], z0[:])  # batched z - z0
        st.update(dzz=dzz, dzneg=dzneg)

    def stage_b2(st, off=0, size=T):
        """res/hazard/intensity matvecs (col strips 0/32/64) + epilogue."""
        i = st["i"]
        pmv = ps.tile([128, T], F32, tag="pmv", bufs=2)
        # k-interleaved back-to-back so the col strips overlap on the PE
        for k in range(KC):
            nc.tensor.matmul(
                pmv[0:1, :size], lhsT=wmv_s[:, k, 0:1],
                rhs=st["Xs"][k][:, off:off + size],
                start=(k == 0), stop=(k == KC - 1),
            )
            nc.tensor.matmul(
                pmv[32:33, :size], lhsT=wmv_s[:, k, 1:2], rhs=st["dzz"][:, k],
                start=(k == 0), stop=(k == KC - 1),
            )
            nc.tensor.matmul(
                pmv[64:65, :size], lhsT=wmv_s[:, k, 2:3], rhs=st["dzneg"][:, k],
                start=(k == 0), stop=(k == KC - 1),
            )

        # epilogue (DVE): res = relu(pres + br), hz = res*t + phaz,
        # it = (res + eps) + pint
        res = sb.tile([1, size], F32, tag="res", bufs=2)
        nc.vector.tensor_scalar(
            res[:], pmv[0:1, :size], br, 0.0, op0=ALU.add, op1=ALU.max
        )
        rt = sb.tile([1, size], F32, tag="rt", bufs=2)
        osl = slice(i * T + off, i * T + off + size)
        nc.vector.tensor_mul(rt[:], res[:], t_all[0:1, osl])
        nc.vector.tensor_add(out_hz[0:1, osl], rt[:], pmv[32:33, :size])
        nc.vector.scalar_tensor_tensor(
            out_it[0:1, osl], in0=res[:], scalar=MINPOS, in1=pmv[64:65, :size],
            op0=ALU.add, op1=ALU.add,
        )
        # stream this tile's outputs (gpsimd queues are idle)
        nc.gpsimd.dma_start(io["hazard"][0:1, osl], out_hz[0:1, osl])
        nc.gpsimd.dma_start(io["intensity"][0:1, osl], out_it[0:1, osl])

    # software pipeline, two tiles in flight:
    # A0 A1 B1(0) A2 [B2(0) B1(1)] A3 [B2(1) B1(2)] ...
    # The last tile's B stages run in token halves to shorten the serial
    # drain chain at the end of the kernel.
    states = [stage_a(0), stage_a(1)]
    stage_b1(states[0])
    for i in range(2, NTILE):
        states.append(stage_a(i))
        stage_b2(states[i - 2])
        stage_b1(states[i - 1])
    stage_b2(states[NTILE - 2])
    last = states[NTILE - 1]
    stage_b1(last, 0, T // 2)
    lastB = dict(last)
    stage_b1(lastB, T // 2, T // 2)
    stage_b2(last, 0, T // 2)
    stage_b2(lastB, T // 2, T // 2)


def build_nc():
    nc = bacc.Bacc(
        "TRN2", target_bir_lowering=False, debug=False, enable_asserts=False
    )
    io = {
        "hbT": nc.dram_tensor("hbT", [KC, 128, NTOK], BF16, kind="ExternalInput").ap(),
        "t": nc.dram_tensor("t", [1, NTOK], F32, kind="ExternalInput").ap(),

        "whTm": nc.dram_tensor(
            "whTm", [KC, 128, KC, 128], BF16, kind="ExternalInput"
        ).ap(),
        "w1T": nc.dram_tensor("w1T", [128, KC, H], BF16, kind="ExternalInput").ap(),
        "wmv": nc.dram_tensor("wmv", [128, KC, 3], BF16, kind="ExternalInput").ap(),
        "wvec": nc.dram_tensor("wvec", [128, 17], F32, kind="ExternalInput").ap(),

        "intensity": nc.dram_tensor(
            "intensity", [1, NTOK], F32, kind="ExternalOutput"
        ).ap(),
        "hazard": nc.dram_tensor(
            "hazard", [1, NTOK], F32, kind="ExternalOutput"
        ).ap(),
    }
    with tile.TileContext(nc) as tc:
        with ExitStack() as ctx:
            _build_body(ctx, tc, io)
    nc.compile()
    return nc


def _get_nc():
    global _NC_CACHE
    if _NC_CACHE is None:
        _NC_CACHE = build_nc()
    return _NC_CACHE


def prep_in_maps(inputs: dict) -> list[dict]:
    """Host-side preprocessing: relu-mask params, transpose/cast, shard."""
    bf = ml_dtypes.bfloat16
    f32 = np.float32

    def arr(name):
        return np.asarray(inputs[name], f32)

    h, t = arr("h"), arr("t")
    relu = lambda x: np.maximum(x, 0.0)
    wt = relu(arr("w_t1"))
    W1p = relu(arr("W1"))
    b1p = relu(arr("b1"))
    W2p = relu(arr("W2"))
    W_r, b_r, b_h1, W_h1 = arr("W_r"), arr("b_r"), arr("b_h1"), arr("W_h1")

    def kchunk(M):  # [512, X] -> [128, KC, X]: [p, k, :] = M[k*128+p, :]
        return np.ascontiguousarray(M.reshape(KC, 128, -1).transpose(1, 0, 2))

    whT = kchunk(W_h1.T.astype(bf))          # [128, KC_k, H]
    # [KC_m, 128, KC_k, 128]: whTm[m][p, k, j] = W_h1.T[k*128+p, m*128+j]
    whTm = np.ascontiguousarray(
        whT.reshape(128, KC, KC, 128).transpose(2, 0, 1, 3)
    )
    w1T = kchunk(W1p.T.astype(bf))
    wmv = kchunk(np.stack([W_r[0], W2p[0], -W2p[0]], axis=1).astype(bf))

    def chunked(v):  # [512] -> [128, KC]
        return v.reshape(KC, 128).T

    wvec = np.zeros((128, 17), f32)
    wvec[:, 0:KC] = chunked(b_h1)
    wvec[:, KC:2 * KC] = chunked(b1p)
    wvec[:, 2 * KC:3 * KC] = chunked(wt)
    wvec[:, 3 * KC:4 * KC] = chunked(-wt)
    wvec[0, 16] = b_r[0]

    shared = {
        "whTm": whTm,
        "w1T": w1T,
        "wmv": wmv,
        "wvec": wvec,
    }
    # pre-transposed chunk-major: hbT[c][k] = h_shard[:, k*128:(k+1)*128].T
    hb_all = np.ascontiguousarray(
        h.reshape(NCORES, NTOK, KC, 128).transpose(0, 2, 3, 1)
    ).astype(bf)
    t_all = t.reshape(NCORES, 1, NTOK).astype(f32)
    return [
        {
            "hbT": np.ascontiguousarray(hb_all[c]),
            "t": np.ascontiguousarray(t_all[c]),
            **shared,
        }
        for c in range(NCORES)
    ]


def run(inputs: dict, trace: bool = False):
    nc = _get_nc()
    in_maps = prep_in_maps(inputs)
    res = bass_utils.run_bass_kernel_spmd(
        nc, in_maps, core_ids=list(range(NCORES)), trace=trace
    )
    intensity = (
        np.concatenate([r["intensity"].reshape(-1) for r in res.results])
        .reshape(B, S)
        .astype(np.float32)
    )
    hazard = (
        np.concatenate([r["hazard"].reshape(-1) for r in res.results])
        .reshape(B, S, 1)
        .astype(np.float32)
    )
    return (intensity, hazard), res


def kernel(**inputs):
    (intensity, hazard), _ = run(inputs)
    return intensity, hazard


# revision 63
# speedup vs baseline: 1.0893x; 1.0893x over previous
"""Trainium2 Bass kernel for nn_Decoder_30949534335472 (hazard-MLP decoder).

Math (per token, H=512):
    h2  = h @ W_h1.T + b_h1
    res = relu(h @ W_r.T + b_r)
    a0  = tanh(h2);            z0 = tanh(a0 @ W1p.T + b1p)
    a   = tanh(h2 + wt*t);     z  = tanh(a @ W1p.T + b1p)
    hazard    = W2p·(z - z0) + res*t          (b2p cancels)
    intensity = W2p·[(1-z^2) ⊙ (W1p @ ((1-a^2) ⊙ wt))] + res + 1e-6
where wt/W1p/b1p/W2p are the relu-masked hazard params (jax.grad replaced
by its closed form).

Sharding: pure data parallel, batch axis 16 -> 8 cores x 2.
Layout: feature-major ([feature 128-chunk, token] tiles); h cast to bf16 on
host, loaded transposed via DMA-transpose (all issued up front on sync).
The t-term of the t-path is folded into the PSUM accumulation as a K=1
matmul (wt ⊗ t), so h2 and u share one bank per chunk.
Engines: PE = 68 MMs + 12 col-tiled matvecs per 512-token tile; ACT = tanh
(+bias fused); DVE = fused gradient/epilogue ops; GpSimd = DMAs only.
"""

import sys

sys.path.insert(0, "/opt/trn_rl_repo")

from contextlib import ExitStack

import ml_dtypes
import numpy as np

import concourse.bass as bass
import concourse.tile as tile
from concourse import bacc, mybir
from concourse import bass_utils

F32 = mybir.dt.float32
BF16 = mybir.dt.bfloat16
FP8 = mybir.dt.float8e4
FP8_W_SCALE = 16.0  # W1p is tiny (~N(0,1/512) masked); scale into fp8 range
AF = mybir.ActivationFunctionType
ALU = mybir.AluOpType

B, S, H = 16, 2048, 512
NCORES = 8
NTOK = (B // NCORES) * S  # 4096 tokens per core
T = 512                   # tokens per compute tile
NTILE = NTOK // T         # 8
TM = 2048                 # tokens per X-load macro slab
NMACRO = NTOK // TM       # 2
KC = H // 128             # 4 feature chunks
MINPOS = 1e-6

_NC_CACHE = None


def _build_body(ctx: ExitStack, tc: "tile.TileContext", io: dict):
    nc = tc.nc

    wpool = ctx.enter_context(tc.tile_pool(name="weights", bufs=1))
    sb = ctx.enter_context(tc.tile_pool(name="sb", bufs=2))
    ps = ctx.enter_context(tc.tile_pool(name="ps", bufs=2, space="PSUM"))

    # ---- weights: packed DMAs on gpsimd (SWDGE), off the transpose path ----
    # per-output-chunk whT tiles: mm1(m=0) only gates on its own 2 DMAs
    whTm = [wpool.tile([128, KC, 128], BF16, name=f"whTm{m}") for m in range(KC)]
    w1T_s = wpool.tile([128, KC, H], BF16)
    wmv_s = wpool.tile([128, KC, 3], BF16)   # cols: W_r, W2p, -W2p
    wvec = wpool.tile([128, 17], F32)        # bh1 | b1p | wt | -wt | b_r@[0,16]
    w18 = wpool.tile([128, KC, H], FP8)      # W1p.T * FP8_W_SCALE, fp8e4m3
    t_all = wpool.tile([1, NTOK], F32)
    # DMA engine split: gpsimd carries only tiny vectors (its
    # partition_broadcast serializes behind in-flight SWDGE transfers);
    # w1T rides the scalar engine's HWDGE (idle until the first ACT);
    # everything else is ordered on sync by when the PE needs it.
    nc.gpsimd.dma_start(t_all[:], io["t"][:])
    nc.gpsimd.dma_start(wvec[:], io["wvec"][:])
    nc.gpsimd.dma_start(wmv_s[:], io["wmv"][:])
    for k in range(KC):
        nc.scalar.dma_start(w1T_s[:, k], io["w1T"][:, k])
    nc.scalar.dma_start(w18[:], io["w18"][:])


    bh1 = wvec[:, 0:KC]
    b1p = wvec[:, KC:2 * KC]
    wt = wvec[:, 2 * KC:3 * KC]
    nwt = wvec[:, 3 * KC:4 * KC]
    br = wvec[0:1, 16:17]

    # ---- X = h^T loads: plain contiguous DMAs (h pre-transposed on host) --
    # The DMA-transpose xbar path serializes (~52 GB/s, ~1.2us issue each);
    # with hbT = [KC, 128, NTOK] from the host these are ordinary strided
    # DMAs with 4KB-contiguous rows that spread across all queues.
    # Graded slab sizes, each split in 2 sub-DMAs so transfers parallelize
    # across HWDGE queues; tile 0's X lands first, then whTm, then the rest.
    Xt = {}  # (tile, chunk) -> AP [128, T]
    slabs = [(0, 512), (512, 512), (1024, 1024), (2048, 2048)]
    nslab = {512: 2, 1024: 1, 2048: 1}

    def emit_slab(start, size):
        for k in range(KC):
            Xm = sb.tile([128, size], BF16, tag=f"Xm{size}", bufs=nslab[size] * KC)
            h_ = size // 2
            nc.sync.dma_start(Xm[:, 0:h_], io["hbT"][k, :, start:start + h_])
            nc.sync.dma_start(Xm[:, h_:], io["hbT"][k, :, start + h_:start + size])
            for i in range(start // T, (start + size) // T):
                off = i * T - start
                Xt[(i, k)] = Xm[:, off:off + T]

    emit_slab(*slabs[0])
    for m in range(KC):
        nc.sync.dma_start(whTm[m][:, 0:2], io["whTm"][m, :, 0:2])
        nc.sync.dma_start(whTm[m][:, 2:4], io["whTm"][m, :, 2:4])
    for sl in slabs[1:]:
        emit_slab(*sl)

    # ---- output accumulators, one DMA each at the end ----
    out_hz = wpool.tile([1, NTOK], F32)
    out_it = wpool.tile([1, NTOK], F32)

    def stage_a(i):
        """mm1 (+bias-free h2 then u via K=1 wt*t MM), a0/a/sq/da, res mv."""
        st = {"i": i}
        Xs = [Xt[(i, k)] for k in range(KC)]
        tB = sb.tile([128, T], F32, tag="tB", bufs=2)
        nc.gpsimd.partition_broadcast(tB[:], t_all[0:1, i * T:(i + 1) * T])

        a0 = sb.tile([128, KC, T], BF16, tag="a0", bufs=2)
        a = sb.tile([128, KC, T], BF16, tag="a", bufs=2)
        da = sb.tile([128, KC, T], FP8, tag="da", bufs=2)
        sq = sb.tile([128, KC, T], BF16, tag="sq", bufs=2)

        for m in range(KC):
            pv1 = ps.tile([128, T], F32, tag="pv1", bufs=2)
            for k in range(KC):
                nc.tensor.matmul(
                    pv1[:],
                    lhsT=whTm[m][:, k],
                    rhs=Xs[k],
                    start=(k == 0),
                    stop=(k == KC - 1),
                )
            nc.scalar.activation(a0[:, m], pv1[:], AF.Tanh, bias=bh1[:, m:m + 1])
            # u = h2 + wt*t on DVE (PE is the bottleneck engine)
            u = sb.tile([128, T], F32, tag="u", bufs=2)
            nc.vector.scalar_tensor_tensor(
                u[:], in0=tB[:], scalar=wt[:, m:m + 1], in1=pv1[:],
                op0=ALU.mult, op1=ALU.add,
            )
            nc.scalar.activation(a[:, m], u[:], AF.Tanh, bias=bh1[:, m:m + 1])

        nc.vector.tensor_mul(sq[:], a[:], a[:])  # batched a^2
        for m in range(KC):
            # da = wt - wt*a^2  (= wt*(1-a^2))
            nc.vector.tensor_scalar(
                da[:, m], sq[:, m], nwt[:, m:m + 1], wt[:, m:m + 1],
                op0=ALU.mult, op1=ALU.add,
            )

        st.update(Xs=Xs, a0=a0, a=a, da=da)
        return st

    def stage_b1(st, off=0, size=T):
        """z-chains (k-interleaved, 3 concurrent banks) + gradient
        elementwise, over tokens [off, off+size) of the tile."""
        tsl = slice(off, off + size)
        z0 = sb.tile([128, KC, size], BF16, tag="z0", bufs=2)
        z = sb.tile([128, KC, size], BF16, tag="z", bufs=2)
        zq = sb.tile([128, KC, size], BF16, tag="zq", bufs=2)
        dzneg = sb.tile([128, KC, size], BF16, tag="dzneg", bufs=2)
        dzz = sb.tile([128, KC, size], BF16, tag="dzz", bufs=2)

        for m in range(KC):
            pv0 = ps.tile([128, T], F32, tag="pmm", bufs=4)
            pv = ps.tile([128, T], F32, tag="pmm", bufs=4)
            pg = ps.tile([128, T], F32, tag="pmm", bufs=4)
            msl = slice(m * 128, (m + 1) * 128)
            for k in range(KC):
                nc.tensor.matmul(
                    pv0[:, :size], lhsT=w1T_s[:, k, msl], rhs=st["a0"][:, k, tsl],
                    start=(k == 0), stop=(k == KC - 1),
                )
                nc.tensor.matmul(
                    pv[:, :size], lhsT=w1T_s[:, k, msl], rhs=st["a"][:, k, tsl],
                    start=(k == 0), stop=(k == KC - 1),
                )
                if k % 2 == 0:
                    # fp8 DoubleRow: one MM covers two adjacent K-chunks
                    nc.tensor.matmul(
                        pg[:, :size], lhsT=w18[:, k:k + 2, msl],
                        rhs=st["da"][:, k:k + 2, tsl],
                        start=(k == 0), stop=(k == KC - 2),
                        perf_mode=mybir.MatmulPerfMode.DoubleRow,
                    )
            nc.scalar.activation(
                z0[:, m], pv0[:, :size], AF.Tanh, bias=b1p[:, m:m + 1]
            )
            nc.scalar.activation(
                z[:, m], pv[:, :size], AF.Tanh, bias=b1p[:, m:m + 1]
            )
            nc.vector.tensor_mul(zq[:, m], z[:, m], z[:, m])
            # (z^2 - 1) * g   (negated dz; folded into -W2p in the matvec)
            nc.vector.scalar_tensor_tensor(
                dzneg[:, m], in0=zq[:, m], scalar=1.0, in1=pg[:, :size],
                op0=ALU.subtract, op1=ALU.mult,
            )
        nc.vector.tensor_sub(dzz[:], z[:], z0[:])  # batched z - z0
        st.update(dzz=dzz, dzneg=dzneg)

    def stage_b2(st, off=0, size=T):
        """res/hazard/intensity matvecs (col strips 0/32/64) + epilogue."""
        i = st["i"]
        pmv = ps.tile([128, T], F32, tag="pmv", bufs=2)
        # k-interleaved back-to-back so the col strips overlap on the PE
        for k in range(KC):
            nc.tensor.matmul(
                pmv[0:1, :size], lhsT=wmv_s[:, k, 0:1],
                rhs=st["Xs"][k][:, off:off + size],
                start=(k == 0), stop=(k == KC - 1),
            )
            nc.tensor.matmul(
                pmv[32:33, :size], lhsT=wmv_s[:, k, 1:2], rhs=st["dzz"][:, k],
                start=(k == 0), stop=(k == KC - 1),
            )
            nc.tensor.matmul(
                pmv[64:65, :size], lhsT=wmv_s[:, k, 2:3], rhs=st["dzneg"][:, k],
                start=(k == 0), stop=(k == KC - 1),
            )

        # epilogue (DVE): res = relu(pres + br), hz = res*t + phaz,
        # it = (res + eps) + pint
        res = sb.tile([1, size], F32, tag="res", bufs=2)
        nc.vector.tensor_scalar(
            res[:], pmv[0:1, :size], br, 0.0, op0=ALU.add, op1=ALU.max
        )
        rt = sb.tile([1, size], F32, tag="rt", bufs=2)
        osl = slice(i * T + off, i * T + off + size)
        nc.vector.tensor_mul(rt[:], res[:], t_all[0:1, osl])
        nc.vector.tensor_add(out_hz[0:1, osl], rt[:], pmv[32:33, :size])
        nc.vector.scalar_tensor_tensor(
            out_it[0:1, osl], in0=res[:], scalar=MINPOS, in1=pmv[64:65, :size],
            op0=ALU.add, op1=ALU.add,
        )
        # stream this tile's outputs (gpsimd queues are idle)
        nc.gpsimd.dma_start(io["hazard"][0:1, osl], out_hz[0:1, osl])
        nc.gpsimd.dma_start(io["intensity"][0:1, osl], out_it[0:1, osl])

    # software pipeline, two tiles in flight:
    # A0 A1 B1(0) A2 [B2(0) B1(1)] A3 [B2(1) B1(2)] ...
    # The last tile's B stages run in token halves to shorten the serial
    # drain chain at the end of the kernel.
    states = [stage_a(0), stage_a(1)]
    stage_b1(states[0])
    for i in range(2, NTILE):
        states.append(stage_a(i))
        stage_b2(states[i - 2])
        stage_b1(states[i - 1])
    stage_b2(states[NTILE - 2])
    last = states[NTILE - 1]
    stage_b1(last, 0, T // 2)
    lastB = dict(last)
    stage_b1(lastB, T // 2, T // 2)
    stage_b2(last, 0, T // 2)
    stage_b2(lastB, T // 2, T // 2)


def build_nc():
    nc = bacc.Bacc(
        "TRN2", target_bir_lowering=False, debug=False, enable_asserts=False
    )
    io = {
        "hbT": nc.dram_tensor("hbT", [KC, 128, NTOK], BF16, kind="ExternalInput").ap(),
        "t": nc.dram_tensor("t", [1, NTOK], F32, kind="ExternalInput").ap(),

        "whTm": nc.dram_tensor(
            "whTm", [KC, 128, KC, 128], BF16, kind="ExternalInput"
        ).ap(),
        "w1T": nc.dram_tensor("w1T", [128, KC, H], BF16, kind="ExternalInput").ap(),
        "w18": nc.dram_tensor("w18", [128, KC, H], FP8, kind="ExternalInput").ap(),
        "wmv": nc.dram_tensor("wmv", [128, KC, 3], BF16, kind="ExternalInput").ap(),
        "wvec": nc.dram_tensor("wvec", [128, 17], F32, kind="ExternalInput").ap(),

        "intensity": nc.dram_tensor(
            "intensity", [1, NTOK], F32, kind="ExternalOutput"
        ).ap(),
        "hazard": nc.dram_tensor(
            "hazard", [1, NTOK], F32, kind="ExternalOutput"
        ).ap(),
    }
    with tile.TileContext(nc) as tc:
        with ExitStack() as ctx:
            _build_body(ctx, tc, io)
    nc.compile()
    return nc


def _get_nc():
    global _NC_CACHE
    if _NC_CACHE is None:
        _NC_CACHE = build_nc()
    return _NC_CACHE


def prep_in_maps(inputs: dict) -> list[dict]:
    """Host-side preprocessing: relu-mask params, transpose/cast, shard."""
    bf = ml_dtypes.bfloat16
    f32 = np.float32

    def arr(name):
        return np.asarray(inputs[name], f32)

    h, t = arr("h"), arr("t")
    relu = lambda x: np.maximum(x, 0.0)
    wt = relu(arr("w_t1"))
    W1p = relu(arr("W1"))
    b1p = relu(arr("b1"))
    W2p = relu(arr("W2"))
    W_r, b_r, b_h1, W_h1 = arr("W_r"), arr("b_r"), arr("b_h1"), arr("W_h1")

    def kchunk(M):  # [512, X] -> [128, KC, X]: [p, k, :] = M[k*128+p, :]
        return np.ascontiguousarray(M.reshape(KC, 128, -1).transpose(1, 0, 2))

    whT = kchunk(W_h1.T.astype(bf))          # [128, KC_k, H]
    # [KC_m, 128, KC_k, 128]: whTm[m][p, k, j] = W_h1.T[k*128+p, m*128+j]
    whTm = np.ascontiguousarray(
        whT.reshape(128, KC, KC, 128).transpose(2, 0, 1, 3)
    )
    w1T = kchunk(W1p.T.astype(bf))
    f8 = ml_dtypes.float8_e4m3
    w18 = kchunk((W1p.T * FP8_W_SCALE).astype(f8))
    # intensity matvec weights absorb the 1/FP8_W_SCALE of the fp8 g-chain
    wmv = kchunk(
        np.stack([W_r[0], W2p[0], -W2p[0] / FP8_W_SCALE], axis=1).astype(bf)
    )

    def chunked(v):  # [512] -> [128, KC]
        return v.reshape(KC, 128).T

    wvec = np.zeros((128, 17), f32)
    wvec[:, 0:KC] = chunked(b_h1)
    wvec[:, KC:2 * KC] = chunked(b1p)
    wvec[:, 2 * KC:3 * KC] = chunked(wt)
    wvec[:, 3 * KC:4 * KC] = chunked(-wt)
    wvec[0, 16] = b_r[0]

    shared = {
        "whTm": whTm,
        "w1T": w1T,
        "w18": w18,
        "wmv": wmv,
        "wvec": wvec,
    }
    # pre-transposed chunk-major: hbT[c][k] = h_shard[:, k*128:(k+1)*128].T
    hb_all = np.ascontiguousarray(
        h.reshape(NCORES, NTOK, KC, 128).transpose(0, 2, 3, 1)
    ).astype(bf)
    t_all = t.reshape(NCORES, 1, NTOK).astype(f32)
    return [
        {
            "hbT": np.ascontiguousarray(hb_all[c]),
            "t": np.ascontiguousarray(t_all[c]),
            **shared,
        }
        for c in range(NCORES)
    ]


def run(inputs: dict, trace: bool = False):
    nc = _get_nc()
    in_maps = prep_in_maps(inputs)
    res = bass_utils.run_bass_kernel_spmd(
        nc, in_maps, core_ids=list(range(NCORES)), trace=trace
    )
    intensity = (
        np.concatenate([r["intensity"].reshape(-1) for r in res.results])
        .reshape(B, S)
        .astype(np.float32)
    )
    hazard = (
        np.concatenate([r["hazard"].reshape(-1) for r in res.results])
        .reshape(B, S, 1)
        .astype(np.float32)
    )
    return (intensity, hazard), res


def kernel(**inputs):
    (intensity, hazard), _ = run(inputs)
    return intensity, hazard
